# revision 5
# baseline (speedup 1.0000x reference)
"""Expert-parallel MoE FFN kernel for Trainium2 (8 NeuronCores, one expert per core).

Host side: routes tokens to experts (dedup per expert, summing duplicate top-k
weights), pads each expert's token list to a common T_PAD, and pre-tiles the
weight matrices into DMA-friendly contiguous blocks.

Device side (per core, expert e):
  h^T = silu(G_e^T X^T) * (U_e^T X^T)        [I, T]   (stage A, fp32r matmuls)
  y   = (h^T)^T-contracted-with D_e * cw      [T, H]   (stage B)
All matmuls run as float32r (tf32-like rounding, 1 cycle/row on the PE vs 4
for plain fp32); accumulation is fp32 in PSUM.
"""
import sys

if "/opt/trn_rl_repo" not in sys.path:
    sys.path.insert(0, "/opt/trn_rl_repo")

import numpy as np

N_TOKENS, TOP_K, N_EXPERTS, HIDDEN, INTER = 4096, 2, 8, 1024, 2048
P = 128
NI = INTER // P          # 16 I-tiles
KH = HIDDEN // P         # 8 H(contraction)-tiles
HC = HIDDEN // 512       # 2 output-column chunks

_CACHE = {}


def _build(t_pad):
    import concourse.bacc as bacc
    import concourse.mybir as mybir
    import concourse.tile as tile

    f32 = mybir.dt.float32
    f32r = mybir.dt.float32r

    nt = t_pad // P          # T tiles of 128
    ntc = t_pad // 512       # T chunks of 512

    nc = bacc.Bacc()
    xt = nc.declare_dram_parameter("xt", [KH, P, t_pad], f32r, isOutput=False)
    gw = nc.declare_dram_parameter("gw", [NI, P, HIDDEN], f32r, isOutput=False)
    uw = nc.declare_dram_parameter("uw", [NI, P, HIDDEN], f32r, isOutput=False)
    dw = nc.declare_dram_parameter("dw", [HC, NI, P, 512], f32r, isOutput=False)
    cw = nc.declare_dram_parameter("cw", [P, nt], f32, isOutput=False)
    y = nc.declare_dram_parameter("y", [t_pad, HIDDEN], f32, isOutput=True)

    with tile.TileContext(nc) as tc:
        with (
            tc.tile_pool(name="xp", bufs=1) as xp,
            tc.tile_pool(name="hp", bufs=1) as hp,
            tc.tile_pool(name="wp", bufs=2) as wp,
            tc.tile_pool(name="dp", bufs=3) as dp,
            tc.tile_pool(name="ep", bufs=3) as ep,
            tc.tile_pool(name="cp", bufs=1) as cp,
        ):
            cwt = cp.tile([P, nt], f32)
            nc.sync.dma_start(out=cwt[:], in_=cw[:])

            xts = []
            for k in range(KH):
                t = xp.tile([P, t_pad], f32r, tag=f"x{k}")
                nc.sync.dma_start(out=t[:], in_=xt[k])
                xts.append(t)

            hts = [hp.tile([P, t_pad], f32r, tag=f"h{i}", name=f"ht{i}") for i in range(NI)]

            # ---- Stage A: h^T[i] = silu(G^T X^T) * (U^T X^T), tiled over I ----
            with tc.tile_pool(name="psA", bufs=2, space="PSUM") as psA:
                for i in range(NI):
                    gt = wp.tile([P, HIDDEN], f32r, tag="g")
                    ut = wp.tile([P, HIDDEN], f32r, tag="u")
                    nc.sync.dma_start(out=gt[:], in_=gw[i])
                    nc.sync.dma_start(out=ut[:], in_=uw[i])
                    pgs = [psA.tile([P, 512], f32, tag=f"pg{c}", name=f"pg{i}_{c}") for c in range(ntc)]
                    pus = [psA.tile([P, 512], f32, tag=f"pu{c}", name=f"pu{i}_{c}") for c in range(ntc)]
                    for k in range(KH):
                        lg = gt[:, k * P:(k + 1) * P]
                        lu = ut[:, k * P:(k + 1) * P]
                        for c in range(ntc):
                            rx = xts[k][:, c * 512:(c + 1) * 512]
                            nc.tensor.matmul(out=pgs[c][:], lhsT=lg, rhs=rx,
                                             start=(k == 0), stop=(k == KH - 1))
                        for c in range(ntc):
                            rx = xts[k][:, c * 512:(c + 1) * 512]
                            nc.tensor.matmul(out=pus[c][:], lhsT=lu, rhs=rx,
                                             start=(k == 0), stop=(k == KH - 1))
                    for c in range(ntc):
                        sg = ep.tile([P, 512], f32, tag="sg")
                        nc.scalar.activation(out=sg[:], in_=pgs[c][:],
                                             func=mybir.ActivationFunctionType.Silu)
                        nc.vector.tensor_mul(out=hts[i][:, c * 512:(c + 1) * 512],
                                             in0=sg[:], in1=pus[c][:])

            # ---- Stage B: y[:, hc] = sum_i h^T[i]^T @ D[i, hc], scaled by cw ----
            with tc.tile_pool(name="psB", bufs=1, space="PSUM") as psB:
                for hc in range(HC):
                    pys = [psB.tile([P, 512], f32, tag=f"py{tt}", name=f"py{hc}_{tt}") for tt in range(nt)]
                    for i in range(NI):
                        dt_ = dp.tile([P, 512], f32r, tag="d")
                        nc.sync.dma_start(out=dt_[:], in_=dw[hc, i])
                        for tt in range(nt):
                            nc.tensor.matmul(out=pys[tt][:],
                                             lhsT=hts[i][:, tt * P:(tt + 1) * P],
                                             rhs=dt_[:],
                                             start=(i == 0), stop=(i == NI - 1))
                    for tt in range(nt):
                        ysb = ep.tile([P, 512], f32, tag="y")
                        nc.scalar.activation(out=ysb[:], in_=pys[tt][:],
                                             func=mybir.ActivationFunctionType.Copy,
                                             scale=cwt[:, tt:tt + 1])
                        nc.gpsimd.dma_start(out=y[tt * P:(tt + 1) * P,
                                                  hc * 512:(hc + 1) * 512],
                                            in_=ysb[:])

    nc.finalize()
    return nc


def _route(expert_indices, expert_weights):
    idx = np.asarray(expert_indices).astype(np.int64)
    wts = np.asarray(expert_weights).astype(np.float32)
    n = idx.shape[0]
    cw_full = np.zeros((N_EXPERTS, n), np.float32)
    for k in range(idx.shape[1]):
        np.add.at(cw_full, (idx[:, k], np.arange(n)), wts[:, k])
    ids = [np.nonzero(cw_full[e])[0] for e in range(N_EXPERTS)]
    maxc = max(len(i) for i in ids)
    t_pad = max(512, ((maxc + 511) // 512) * 512)
    return cw_full, ids, t_pad


_LDW_PATCHED = False


def _patch_ldw_opt():
    """Enable walrus's LDWEIGHTS dedup pass: consecutive matmuls that reuse the
    same stationary tile then skip the redundant ~190ns weight reload."""
    global _LDW_PATCHED
    if _LDW_PATCHED:
        return
    import concourse.bass_utils as bu

    orig = bu.run_command

    def run_command(argv, **kw):
        argv = ["--enable-ldw-opt=true" if a == "--enable-ldw-opt=false" else a
                for a in argv]
        return orig(argv, **kw)

    bu.run_command = run_command
    _LDW_PATCHED = True


def _run(nc, in_maps, trace=False, trace_cores=None):
    _patch_ldw_opt()
    from concourse.bass_utils import run_bass_kernel_spmd

    return run_bass_kernel_spmd(
        nc, in_maps, list(range(N_EXPERTS)), trace=trace,
        trace_cores=trace_cores,
    )


def prepare(tokens, expert_indices, expert_weights, gate_weight, up_weight,
            down_weight):
    """Host-side routing + layout. Returns (nc, in_maps, ids, t_pad)."""
    tokens = np.ascontiguousarray(np.asarray(tokens, dtype=np.float32))
    gate_weight = np.asarray(gate_weight, dtype=np.float32)
    up_weight = np.asarray(up_weight, dtype=np.float32)
    down_weight = np.asarray(down_weight, dtype=np.float32)

    cw_full, ids, t_pad = _route(expert_indices, expert_weights)
    nt = t_pad // P

    key = t_pad
    if key not in _CACHE:
        _CACHE[key] = _build(t_pad)
    nc = _CACHE[key]

    in_maps = []
    for e in range(N_EXPERTS):
        ce = len(ids[e])
        xe = np.zeros((HIDDEN, t_pad), np.float32)
        xe[:, :ce] = tokens[ids[e]].T
        cwe = np.zeros((t_pad,), np.float32)
        cwe[:ce] = cw_full[e, ids[e]]
        in_maps.append({
            "xt": np.ascontiguousarray(xe.reshape(KH, P, t_pad)),
            "gw": np.ascontiguousarray(
                gate_weight[e].reshape(KH, P, NI, P).transpose(2, 1, 0, 3)
            ).reshape(NI, P, HIDDEN),
            "uw": np.ascontiguousarray(
                up_weight[e].reshape(KH, P, NI, P).transpose(2, 1, 0, 3)
            ).reshape(NI, P, HIDDEN),
            "dw": np.ascontiguousarray(
                down_weight[e].reshape(NI, P, HC, 512).transpose(2, 0, 1, 3)),
            "cw": np.ascontiguousarray(cwe.reshape(nt, P).T),
        })
    return nc, in_maps, ids, t_pad


def combine(results, ids):
    out = np.zeros((N_TOKENS, HIDDEN), np.float32)
    for e in range(N_EXPERTS):
        ce = len(ids[e])
        out[ids[e]] += results[e]["y"][:ce]
    return out


def kernel(tokens, expert_indices, expert_weights, gate_weight, up_weight,
           down_weight):
    nc, in_maps, ids, _ = prepare(tokens, expert_indices, expert_weights,
                                  gate_weight, up_weight, down_weight)
    res = _run(nc, in_maps, trace=False)
    return combine(res.results, ids)


# revision 8
# speedup vs baseline: 1.0801x; 1.0801x over previous
"""Expert-parallel MoE FFN kernel for Trainium2 (8 NeuronCores, one expert per core).

Host side: routes tokens to experts (dedup per expert, summing duplicate top-k
weights), pads each expert's token list to a common T_PAD, and pre-tiles the
weight matrices into DMA-friendly contiguous blocks.

Device side (per core, expert e):
  h^T = silu(G_e^T X^T) * (U_e^T X^T)        [I, T]   (stage A, fp32r matmuls)
  y   = (h^T)^T-contracted-with D_e * cw      [T, H]   (stage B)
All matmuls run as float32r (tf32-like rounding, 1 cycle/row on the PE vs 4
for plain fp32); accumulation is fp32 in PSUM.
"""
import sys

if "/opt/trn_rl_repo" not in sys.path:
    sys.path.insert(0, "/opt/trn_rl_repo")

import numpy as np

N_TOKENS, TOP_K, N_EXPERTS, HIDDEN, INTER = 4096, 2, 8, 1024, 2048
P = 128
NI = INTER // P          # 16 I-tiles
KH = HIDDEN // P         # 8 H(contraction)-tiles
HC = HIDDEN // 512       # 2 output-column chunks

_CACHE = {}


def _build(t_pad):
    import concourse.bacc as bacc
    import concourse.mybir as mybir
    import concourse.tile as tile

    f32 = mybir.dt.float32
    f32r = mybir.dt.float32r

    nt = t_pad // P          # T tiles of 128
    ntc = t_pad // 512       # T chunks of 512

    nc = bacc.Bacc()
    xt = nc.declare_dram_parameter("xt", [KH, P, t_pad], f32r, isOutput=False)
    gw = nc.declare_dram_parameter("gw", [NI, P, HIDDEN], f32r, isOutput=False)
    uw = nc.declare_dram_parameter("uw", [NI, P, HIDDEN], f32r, isOutput=False)
    dw = nc.declare_dram_parameter("dw", [NI, P, HIDDEN], f32r, isOutput=False)
    cw = nc.declare_dram_parameter("cw", [P, t_pad], f32, isOutput=False)
    y = nc.declare_dram_parameter("y", [HIDDEN, t_pad], f32, isOutput=True)

    with tile.TileContext(nc) as tc:
        with (
            tc.tile_pool(name="hp", bufs=1) as hp,
            tc.tile_pool(name="wp", bufs=2) as wp,
            tc.tile_pool(name="ep", bufs=3) as ep,
            tc.tile_pool(name="cp", bufs=1) as cp,
        ):
            cwt = cp.tile([P, t_pad], f32)
            nc.sync.dma_start(out=cwt[:], in_=cw[:])

            hts = [hp.tile([P, t_pad], f32r, tag=f"h{i}", name=f"ht{i}") for i in range(NI)]

            # ---- Stage A: h^T[i] = silu(G^T X^T) * (U^T X^T), tiled over I ----
            with (
                tc.tile_pool(name="xp", bufs=1) as xp,
                tc.tile_pool(name="psA", bufs=2, space="PSUM") as psA,
            ):
                xts = []
                for k in range(KH):
                    t = xp.tile([P, t_pad], f32r, tag=f"x{k}")
                    nc.sync.dma_start(out=t[:], in_=xt[k])
                    xts.append(t)
                for i in range(NI):
                    gt = wp.tile([P, HIDDEN], f32r, tag="g")
                    ut = wp.tile([P, HIDDEN], f32r, tag="u")
                    nc.sync.dma_start(out=gt[:], in_=gw[i])
                    nc.sync.dma_start(out=ut[:], in_=uw[i])
                    pgs = [psA.tile([P, 512], f32, tag=f"pg{c}", name=f"pg{i}_{c}") for c in range(ntc)]
                    pus = [psA.tile([P, 512], f32, tag=f"pu{c}", name=f"pu{i}_{c}") for c in range(ntc)]
                    for k in range(KH):
                        lg = gt[:, k * P:(k + 1) * P]
                        lu = ut[:, k * P:(k + 1) * P]
                        for c in range(ntc):
                            rx = xts[k][:, c * 512:(c + 1) * 512]
                            nc.tensor.matmul(out=pgs[c][:], lhsT=lg, rhs=rx,
                                             start=(k == 0), stop=(k == KH - 1))
                        for c in range(ntc):
                            rx = xts[k][:, c * 512:(c + 1) * 512]
                            nc.tensor.matmul(out=pus[c][:], lhsT=lu, rhs=rx,
                                             start=(k == 0), stop=(k == KH - 1))
                    for c in range(ntc):
                        sg = ep.tile([P, 512], f32, tag="sg")
                        nc.scalar.activation(out=sg[:], in_=pgs[c][:],
                                             func=mybir.ActivationFunctionType.Silu)
                        nc.vector.tensor_mul(out=hts[i][:, c * 512:(c + 1) * 512],
                                             in0=sg[:], in1=pus[c][:])

            # ---- Stage B: y^T[j,:] = sum_i D[i,j-cols]^T @ h^T[i], * cw ----
            # dw tile is the stationary operand: one weight load serves ntc
            # matmuls. Output is y^T [H, T]; host transposes back.
            jg = max(1, 8 // ntc)          # j-tiles per group, jg*ntc <= 8 banks
            with (
                tc.tile_pool(name="dwp", bufs=1) as dwp,
                tc.tile_pool(name="psB", bufs=1, space="PSUM") as psB,
            ):
                dts = []
                for i in range(NI):
                    dt_ = dwp.tile([P, HIDDEN], f32r, tag=f"d{i}", name=f"dt{i}")
                    nc.sync.dma_start(out=dt_[:], in_=dw[i])
                    dts.append(dt_)
                for j0 in range(0, KH, jg):
                    pys = [psB.tile([P, 512], f32, tag=f"py{jj}_{c}",
                                    name=f"py{j0}_{jj}_{c}")
                           for jj in range(jg) for c in range(ntc)]
                    for i in range(NI):
                        for jj in range(jg):
                            ld = dts[i][:, (j0 + jj) * P:(j0 + jj + 1) * P]
                            for c in range(ntc):
                                nc.tensor.matmul(out=pys[jj * ntc + c][:],
                                                 lhsT=ld,
                                                 rhs=hts[i][:, c * 512:(c + 1) * 512],
                                                 start=(i == 0), stop=(i == NI - 1))
                    for jj in range(jg):
                        for c in range(ntc):
                            ysb = ep.tile([P, 512], f32, tag="y")
                            nc.vector.tensor_mul(out=ysb[:],
                                                 in0=pys[jj * ntc + c][:],
                                                 in1=cwt[:, c * 512:(c + 1) * 512])
                            nc.gpsimd.dma_start(
                                out=y[(j0 + jj) * P:(j0 + jj + 1) * P,
                                      c * 512:(c + 1) * 512],
                                in_=ysb[:])

    nc.finalize()
    return nc


def _route(expert_indices, expert_weights):
    idx = np.asarray(expert_indices).astype(np.int64)
    wts = np.asarray(expert_weights).astype(np.float32)
    n = idx.shape[0]
    cw_full = np.zeros((N_EXPERTS, n), np.float32)
    for k in range(idx.shape[1]):
        np.add.at(cw_full, (idx[:, k], np.arange(n)), wts[:, k])
    ids = [np.nonzero(cw_full[e])[0] for e in range(N_EXPERTS)]
    maxc = max(len(i) for i in ids)
    t_pad = max(512, ((maxc + 511) // 512) * 512)
    return cw_full, ids, t_pad


_LDW_PATCHED = False


def _patch_ldw_opt():
    """Enable walrus's LDWEIGHTS dedup pass: consecutive matmuls that reuse the
    same stationary tile then skip the redundant ~190ns weight reload."""
    global _LDW_PATCHED
    if _LDW_PATCHED:
        return
    import concourse.bass_utils as bu

    orig = bu.run_command

    def run_command(argv, **kw):
        argv = ["--enable-ldw-opt=true" if a == "--enable-ldw-opt=false" else a
                for a in argv]
        return orig(argv, **kw)

    bu.run_command = run_command
    _LDW_PATCHED = True


def _run(nc, in_maps, trace=False, trace_cores=None):
    from concourse.bass_utils import run_bass_kernel_spmd

    return run_bass_kernel_spmd(
        nc, in_maps, list(range(N_EXPERTS)), trace=trace,
        trace_cores=trace_cores,
    )


def prepare(tokens, expert_indices, expert_weights, gate_weight, up_weight,
            down_weight):
    """Host-side routing + layout. Returns (nc, in_maps, ids, t_pad)."""
    tokens = np.ascontiguousarray(np.asarray(tokens, dtype=np.float32))
    gate_weight = np.asarray(gate_weight, dtype=np.float32)
    up_weight = np.asarray(up_weight, dtype=np.float32)
    down_weight = np.asarray(down_weight, dtype=np.float32)

    cw_full, ids, t_pad = _route(expert_indices, expert_weights)
    nt = t_pad // P

    key = t_pad
    if key not in _CACHE:
        _CACHE[key] = _build(t_pad)
    nc = _CACHE[key]

    in_maps = []
    for e in range(N_EXPERTS):
        ce = len(ids[e])
        xe = np.zeros((HIDDEN, t_pad), np.float32)
        xe[:, :ce] = tokens[ids[e]].T
        cwe = np.zeros((t_pad,), np.float32)
        cwe[:ce] = cw_full[e, ids[e]]
        in_maps.append({
            "xt": np.ascontiguousarray(xe.reshape(KH, P, t_pad)),
            "gw": np.ascontiguousarray(
                gate_weight[e].reshape(KH, P, NI, P).transpose(2, 1, 0, 3)
            ).reshape(NI, P, HIDDEN),
            "uw": np.ascontiguousarray(
                up_weight[e].reshape(KH, P, NI, P).transpose(2, 1, 0, 3)
            ).reshape(NI, P, HIDDEN),
            "dw": np.ascontiguousarray(down_weight[e].reshape(NI, P, HIDDEN)),
            "cw": np.ascontiguousarray(
                np.broadcast_to(cwe[None, :], (P, t_pad))),
        })
    return nc, in_maps, ids, t_pad


def combine(results, ids):
    out = np.zeros((N_TOKENS, HIDDEN), np.float32)
    for e in range(N_EXPERTS):
        ce = len(ids[e])
        out[ids[e]] += results[e]["y"].T[:ce]
    return out


def kernel(tokens, expert_indices, expert_weights, gate_weight, up_weight,
           down_weight):
    nc, in_maps, ids, _ = prepare(tokens, expert_indices, expert_weights,
                                  gate_weight, up_weight, down_weight)
    res = _run(nc, in_maps, trace=False)
    return combine(res.results, ids)


# revision 9
# speedup vs baseline: 1.1806x; 1.0930x over previous
"""Expert-parallel MoE FFN kernel for Trainium2 (8 NeuronCores, one expert per core).

Host side: routes tokens to experts (dedup per expert, summing duplicate top-k
weights), pads each expert's token list to a common T_PAD, and pre-tiles the
weight matrices into DMA-friendly contiguous blocks.

Device side (per core, expert e):
  h^T = silu(G_e^T X^T) * (U_e^T X^T)        [I, T]   (stage A, fp32r matmuls)
  y   = (h^T)^T-contracted-with D_e * cw      [T, H]   (stage B)
All matmuls run as float32r (tf32-like rounding, 1 cycle/row on the PE vs 4
for plain fp32); accumulation is fp32 in PSUM.
"""
import sys

if "/opt/trn_rl_repo" not in sys.path:
    sys.path.insert(0, "/opt/trn_rl_repo")

import numpy as np

N_TOKENS, TOP_K, N_EXPERTS, HIDDEN, INTER = 4096, 2, 8, 1024, 2048
P = 128
NI = INTER // P          # 16 I-tiles
KH = HIDDEN // P         # 8 H(contraction)-tiles
HC = HIDDEN // 512       # 2 output-column chunks

_CACHE = {}
MM_BF16 = True


def _build(t_pad):
    import concourse.bacc as bacc
    import concourse.mybir as mybir
    import concourse.tile as tile

    f32 = mybir.dt.float32
    f32r = mybir.dt.bfloat16 if MM_BF16 else mybir.dt.float32r

    nt = t_pad // P          # T tiles of 128
    ntc = t_pad // 512       # T chunks of 512

    nc = bacc.Bacc()
    xt = nc.declare_dram_parameter("xt", [KH, P, t_pad], f32r, isOutput=False)
    gw = nc.declare_dram_parameter("gw", [NI, P, HIDDEN], f32r, isOutput=False)
    uw = nc.declare_dram_parameter("uw", [NI, P, HIDDEN], f32r, isOutput=False)
    dw = nc.declare_dram_parameter("dw", [NI, P, HIDDEN], f32r, isOutput=False)
    cw = nc.declare_dram_parameter("cw", [P, t_pad], f32, isOutput=False)
    y = nc.declare_dram_parameter("y", [HIDDEN, t_pad], f32, isOutput=True)

    with tile.TileContext(nc) as tc:
        with (
            tc.tile_pool(name="hp", bufs=1) as hp,
            tc.tile_pool(name="wp", bufs=2) as wp,
            tc.tile_pool(name="ep", bufs=3) as ep,
            tc.tile_pool(name="cp", bufs=1) as cp,
        ):
            cwt = cp.tile([P, t_pad], f32)
            nc.sync.dma_start(out=cwt[:], in_=cw[:])

            hts = [hp.tile([P, t_pad], f32r, tag=f"h{i}", name=f"ht{i}") for i in range(NI)]

            # ---- Stage A: h^T[i] = silu(G^T X^T) * (U^T X^T), tiled over I ----
            with (
                tc.tile_pool(name="xp", bufs=1) as xp,
                tc.tile_pool(name="psA", bufs=2, space="PSUM") as psA,
            ):
                xts = []
                for k in range(KH):
                    t = xp.tile([P, t_pad], f32r, tag=f"x{k}")
                    nc.sync.dma_start(out=t[:], in_=xt[k])
                    xts.append(t)
                for i in range(NI):
                    gt = wp.tile([P, HIDDEN], f32r, tag="g")
                    ut = wp.tile([P, HIDDEN], f32r, tag="u")
                    nc.sync.dma_start(out=gt[:], in_=gw[i])
                    nc.sync.dma_start(out=ut[:], in_=uw[i])
                    pgs = [psA.tile([P, 512], f32, tag=f"pg{c}", name=f"pg{i}_{c}") for c in range(ntc)]
                    pus = [psA.tile([P, 512], f32, tag=f"pu{c}", name=f"pu{i}_{c}") for c in range(ntc)]
                    for k in range(KH):
                        lg = gt[:, k * P:(k + 1) * P]
                        lu = ut[:, k * P:(k + 1) * P]
                        for c in range(ntc):
                            rx = xts[k][:, c * 512:(c + 1) * 512]
                            nc.tensor.matmul(out=pgs[c][:], lhsT=lg, rhs=rx,
                                             start=(k == 0), stop=(k == KH - 1))
                        for c in range(ntc):
                            rx = xts[k][:, c * 512:(c + 1) * 512]
                            nc.tensor.matmul(out=pus[c][:], lhsT=lu, rhs=rx,
                                             start=(k == 0), stop=(k == KH - 1))
                    for c in range(ntc):
                        sg = ep.tile([P, 512], f32, tag="sg")
                        nc.scalar.activation(out=sg[:], in_=pgs[c][:],
                                             func=mybir.ActivationFunctionType.Silu)
                        nc.vector.tensor_mul(out=hts[i][:, c * 512:(c + 1) * 512],
                                             in0=sg[:], in1=pus[c][:])

            # ---- Stage B: y^T[j,:] = sum_i D[i,j-cols]^T @ h^T[i], * cw ----
            # dw tile is the stationary operand: one weight load serves ntc
            # matmuls. Output is y^T [H, T]; host transposes back.
            jg = max(1, 8 // ntc)          # j-tiles per group, jg*ntc <= 8 banks
            with (
                tc.tile_pool(name="dwp", bufs=1) as dwp,
                tc.tile_pool(name="psB", bufs=1, space="PSUM") as psB,
            ):
                dts = []
                for i in range(NI):
                    dt_ = dwp.tile([P, HIDDEN], f32r, tag=f"d{i}", name=f"dt{i}")
                    nc.sync.dma_start(out=dt_[:], in_=dw[i])
                    dts.append(dt_)
                for j0 in range(0, KH, jg):
                    pys = [psB.tile([P, 512], f32, tag=f"py{jj}_{c}",
                                    name=f"py{j0}_{jj}_{c}")
                           for jj in range(jg) for c in range(ntc)]
                    for i in range(NI):
                        for jj in range(jg):
                            ld = dts[i][:, (j0 + jj) * P:(j0 + jj + 1) * P]
                            for c in range(ntc):
                                nc.tensor.matmul(out=pys[jj * ntc + c][:],
                                                 lhsT=ld,
                                                 rhs=hts[i][:, c * 512:(c + 1) * 512],
                                                 start=(i == 0), stop=(i == NI - 1))
                    for jj in range(jg):
                        for c in range(ntc):
                            ysb = ep.tile([P, 512], f32, tag="y")
                            nc.vector.tensor_mul(out=ysb[:],
                                                 in0=pys[jj * ntc + c][:],
                                                 in1=cwt[:, c * 512:(c + 1) * 512])
                            nc.gpsimd.dma_start(
                                out=y[(j0 + jj) * P:(j0 + jj + 1) * P,
                                      c * 512:(c + 1) * 512],
                                in_=ysb[:])

    nc.finalize()
    return nc


def _route(expert_indices, expert_weights):
    idx = np.asarray(expert_indices).astype(np.int64)
    wts = np.asarray(expert_weights).astype(np.float32)
    n = idx.shape[0]
    cw_full = np.zeros((N_EXPERTS, n), np.float32)
    for k in range(idx.shape[1]):
        np.add.at(cw_full, (idx[:, k], np.arange(n)), wts[:, k])
    ids = [np.nonzero(cw_full[e])[0] for e in range(N_EXPERTS)]
    maxc = max(len(i) for i in ids)
    t_pad = max(512, ((maxc + 511) // 512) * 512)
    return cw_full, ids, t_pad


_LDW_PATCHED = False


def _patch_ldw_opt():
    """Enable walrus's LDWEIGHTS dedup pass: consecutive matmuls that reuse the
    same stationary tile then skip the redundant ~190ns weight reload."""
    global _LDW_PATCHED
    if _LDW_PATCHED:
        return
    import concourse.bass_utils as bu

    orig = bu.run_command

    def run_command(argv, **kw):
        argv = ["--enable-ldw-opt=true" if a == "--enable-ldw-opt=false" else a
                for a in argv]
        return orig(argv, **kw)

    bu.run_command = run_command
    _LDW_PATCHED = True


def _run(nc, in_maps, trace=False, trace_cores=None):
    from concourse.bass_utils import run_bass_kernel_spmd

    return run_bass_kernel_spmd(
        nc, in_maps, list(range(N_EXPERTS)), trace=trace,
        trace_cores=trace_cores,
    )


def prepare(tokens, expert_indices, expert_weights, gate_weight, up_weight,
            down_weight):
    """Host-side routing + layout. Returns (nc, in_maps, ids, t_pad)."""
    tokens = np.ascontiguousarray(np.asarray(tokens, dtype=np.float32))
    gate_weight = np.asarray(gate_weight, dtype=np.float32)
    up_weight = np.asarray(up_weight, dtype=np.float32)
    down_weight = np.asarray(down_weight, dtype=np.float32)

    cw_full, ids, t_pad = _route(expert_indices, expert_weights)
    nt = t_pad // P

    key = t_pad
    if key not in _CACHE:
        _CACHE[key] = _build(t_pad)
    nc = _CACHE[key]

    in_maps = []
    for e in range(N_EXPERTS):
        ce = len(ids[e])
        xe = np.zeros((HIDDEN, t_pad), np.float32)
        xe[:, :ce] = tokens[ids[e]].T
        cwe = np.zeros((t_pad,), np.float32)
        cwe[:ce] = cw_full[e, ids[e]]
        mmdt = np.dtype("bfloat16") if MM_BF16 else np.float32
        in_maps.append({
            "xt": np.ascontiguousarray(xe.reshape(KH, P, t_pad)).astype(mmdt),
            "gw": np.ascontiguousarray(
                gate_weight[e].reshape(KH, P, NI, P).transpose(2, 1, 0, 3)
            ).reshape(NI, P, HIDDEN).astype(mmdt),
            "uw": np.ascontiguousarray(
                up_weight[e].reshape(KH, P, NI, P).transpose(2, 1, 0, 3)
            ).reshape(NI, P, HIDDEN).astype(mmdt),
            "dw": np.ascontiguousarray(down_weight[e].reshape(NI, P, HIDDEN)).astype(mmdt),
            "cw": np.ascontiguousarray(
                np.broadcast_to(cwe[None, :], (P, t_pad))),
        })
    return nc, in_maps, ids, t_pad


def combine(results, ids):
    out = np.zeros((N_TOKENS, HIDDEN), np.float32)
    for e in range(N_EXPERTS):
        ce = len(ids[e])
        out[ids[e]] += results[e]["y"].T[:ce]
    return out


def kernel(tokens, expert_indices, expert_weights, gate_weight, up_weight,
           down_weight):
    nc, in_maps, ids, _ = prepare(tokens, expert_indices, expert_weights,
                                  gate_weight, up_weight, down_weight)
    res = _run(nc, in_maps, trace=False)
    return combine(res.results, ids)
